# revision 1
# baseline (speedup 1.0000x reference)
"""Distributed AlphaFold-style triangle multiplication ("outgoing") on 8
Trainium2 NeuronCores, written in Bass/Tile.

Strategy (sharding_hint followed): the 2*C projection channels are sharded
across the 8 cores for the per-channel [768x768]@[768x768]^T einsum, with
token-sharded (sequence-parallel over the contraction axis) layernorm +
projections before it, linked by an AllToAll, and a second AllToAll back to
token sharding for the output layernorm/projection/gating.
"""
import sys
sys.path.insert(0, "/opt/trn_rl_repo")
import numpy as np
import ml_dtypes
from contextlib import ExitStack

import concourse.bass as bass
import concourse.tile as tile
from concourse import mybir
from concourse.bass_utils import run_bass_kernel_spmd

NCORES = 8
N = 768
C = 128
TB = N // NCORES            # 96 t2-rows per rank
TOK = N * TB                # 73728 tokens per rank
CH1 = 512                   # P1 chunk tokens
NCH1 = TOK // CH1           # 144
NQ = 4                      # A2A#1 token quarters
TOKQ = TOK // NQ            # 18432
CHQ = NCH1 // NQ            # 36 chunks per quarter
TBQ = TB // NQ              # 24 t2-rows per quarter
NG = 4                      # A2A#2 channel groups
CPG = 16 // NG              # 4 channels per group
CH4 = 384                   # P4 chunk tokens (divides 768)
NCH4 = TOK // CH4           # 192
dt = mybir.dt
F32, BF16 = dt.float32, dt.bfloat16
AL = mybir.AluOpType
AF = mybir.ActivationFunctionType


def split_excess_waits(nc, max_waits=1):
    cnt = 0
    for fn in nc.m.functions:
        for bb in fn.blocks:
            insts = list(bb.instructions)
            out = []
            changed = False
            for inst in insts:
                si = inst.sync_info
                if si is not None and si.on_wait and len(si.on_wait) > max_waits:
                    waits = list(si.on_wait)
                    extra, keep = waits[:-max_waits], waits[-max_waits:]
                    for j in range(0, len(extra), max_waits):
                        out.append(mybir.InstNoOp(
                            name=f"{inst.name}_wsplit{j}", ins=[], outs=[],
                            sync_info=mybir.SyncInfo(on_wait=extra[j:j + max_waits], on_update=[]),
                            engine=inst.engine))
                        cnt += 1
                    si.on_wait = keep
                    changed = True
                out.append(inst)
            if changed:
                bb.instructions = out
    return cnt


def build_nc():
    nc = bass.Bass("TRN2", target_bir_lowering=False, debug=False, num_devices=NCORES)

    actT = nc.declare_dram_parameter("actT", [C, TOK], BF16, isOutput=False)
    maskT = nc.declare_dram_parameter("maskT", [1, TOK], BF16, isOutput=False)
    # 5 stationary lhsT weights [c, d]: wpa, wpb, wga, wgb, wgl
    wstack = nc.declare_dram_parameter("wstack", [C, 5 * C], BF16, isOutput=False)
    woT = nc.declare_dram_parameter("woT", [C, C], BF16, isOutput=False)
    # small fp32 columns: [cga, cgb, cgl, wso, co]
    cols = nc.declare_dram_parameter("cols", [C, 5], F32, isOutput=False)
    outT = nc.declare_dram_parameter("outT", [C, TOK], BF16, isOutput=True)

    with tile.TileContext(nc) as tc, ExitStack() as ctx:
        dram = ctx.enter_context(tc.tile_pool(name="dram", bufs=1, space="DRAM"))
        wpool = ctx.enter_context(tc.tile_pool(name="wpool", bufs=1))

        # persistent DRAM intermediates
        p_src_q = [dram.tile([256, TOKQ], BF16, name=f"p_src{q}") for q in range(NQ)]
        p_dst_q = [dram.tile([256, TOKQ], BF16, name=f"p_dst{q}") for q in range(NQ)]
        tri_src_g = [dram.tile([N, CPG, N], BF16, name=f"tri_src{g}") for g in range(NG)]
        tri_dst_g = [dram.tile([NCORES, TB, CPG, N], BF16, name=f"tri_dst{g}") for g in range(NG)]
        gT = dram.tile([C, TOK], BF16, name="gT")

        # persistent SBUF constants
        wst = wpool.tile([C, 5 * C], BF16)
        nc.sync.dma_start(wst[:], wstack[:, :])
        wo_t = wpool.tile([C, C], BF16)
        nc.sync.dma_start(wo_t[:], woT[:, :])
        colst = wpool.tile([C, 5], F32)
        nc.sync.dma_start(colst[:], cols[:, :])
        cga, cgb, cgl, wso, co = (colst[:, i:i + 1] for i in range(5))
        ones_col = wpool.tile([C, 1], BF16)      # 1/128 for mean reduce
        nc.vector.memset(ones_col[:], 1.0 / 128.0)
        ones_row = wpool.tile([1, C], BF16)      # 1.0 for partition broadcast
        nc.vector.memset(ones_row[:], 1.0)

        # ---------------- Phase 1 ----------------
        with tc.tile_pool(name="p1sb", bufs=3) as sb, \
             tc.tile_pool(name="p1ps_s", bufs=3, space="PSUM") as ps_s, \
             tc.tile_pool(name="p1ps_p", bufs=5, space="PSUM") as ps_p:
            for ci in range(NCH1):
                q, cq = ci // CHQ, ci % CHQ
                t0 = ci * CH1
                a16 = sb.tile([C, CH1], BF16, tag="a16")
                nc.sync.dma_start(a16[:], actT[:, t0:t0 + CH1])
                mask_b = sb.tile([C, CH1], BF16, tag="mask_b")
                nc.sync.dma_start(mask_b[:], maskT[:, t0:t0 + CH1].to_broadcast((C, CH1)))

                sq16 = sb.tile([C, CH1], BF16, tag="sq16")
                nc.scalar.square(sq16[:], a16[:])
                s1 = ps_s.tile([1, CH1], F32, tag="stat")
                nc.tensor.matmul(s1[:], ones_col[:], a16[:], start=True, stop=True)
                s2 = ps_s.tile([1, CH1], F32, tag="stat")
                nc.tensor.matmul(s2[:], ones_col[:], sq16[:], start=True, stop=True)

                ss = sb.tile([1, CH1], F32, tag="ss")
                nc.vector.tensor_mul(ss[:], s1[:], s1[:])
                var = sb.tile([1, CH1], F32, tag="var")
                nc.vector.tensor_tensor(var[:], s2[:], ss[:], op=AL.subtract)
                nc.vector.tensor_scalar_add(var[:], var[:], 1e-5)
                vr = sb.tile([1, CH1], F32, tag="vr")
                nc.vector.reciprocal(vr[:], var[:])
                rstd16 = sb.tile([1, CH1], BF16, tag="rstd16")
                nc.scalar.sqrt(rstd16[:], vr[:])
                nrsm16 = sb.tile([1, CH1], BF16, tag="nrsm16")
                nc.vector.scalar_tensor_tensor(
                    nrsm16[:], in0=s1[:], scalar=-1.0, in1=rstd16[:],
                    op0=AL.mult, op1=AL.mult)

                bc_r = ps_s.tile([C, CH1], F32, tag="bc")
                nc.tensor.matmul(bc_r[:], ones_row[:], rstd16[:], start=True, stop=True)
                bc_n = ps_s.tile([C, CH1], F32, tag="bc")
                nc.tensor.matmul(bc_n[:], ones_row[:], nrsm16[:], start=True, stop=True)

                t16 = sb.tile([C, CH1], BF16, tag="t16")
                nc.vector.tensor_mul(t16[:], a16[:], bc_r[:])
                x16 = sb.tile([C, CH1], BF16, tag="x16")
                nc.vector.tensor_tensor(x16[:], t16[:], bc_n[:], op=AL.add)
                xm16 = sb.tile([C, CH1], BF16, tag="xm16")
                nc.gpsimd.tensor_mul(xm16[:], x16[:], mask_b[:])

                pp = {}
                for wi, (nm, rhs) in enumerate(
                        [("pa", xm16), ("pb", xm16), ("ga", x16), ("gb", x16), ("gl", x16)]):
                    ps = ps_p.tile([C, CH1], F32, tag="proj")
                    nc.tensor.matmul(ps[:], wst[:, wi * C:(wi + 1) * C], rhs[:],
                                     start=True, stop=True)
                    pp[nm] = ps
                sa16 = sb.tile([C, CH1], BF16, tag="sa16")
                nc.scalar.activation(sa16[:], pp["ga"][:], AF.Sigmoid, bias=cga)
                sb16 = sb.tile([C, CH1], BF16, tag="sb16")
                nc.scalar.activation(sb16[:], pp["gb"][:], AF.Sigmoid, bias=cgb)
                g16 = sb.tile([C, CH1], BF16, tag="g16")
                nc.scalar.activation(g16[:], pp["gl"][:], AF.Sigmoid, bias=cgl)
                pa16 = sb.tile([C, CH1], BF16, tag="pa16")
                nc.vector.tensor_mul(pa16[:], pp["pa"][:], sa16[:])
                pb16 = sb.tile([C, CH1], BF16, tag="pb16")
                nc.vector.tensor_mul(pb16[:], pp["pb"][:], sb16[:])

                # scatter to p_src_q rows: a-chan d -> row 32*(d//16) + d%16 (+16 for b)
                tq0 = cq * CH1
                dsta = p_src_q[q][:].rearrange("(s k) t -> s k t", s=NCORES)
                nc.sync.dma_start(dsta[:, 0:16, tq0:tq0 + CH1],
                                  pa16[:].rearrange("(s k) t -> s k t", s=8))
                nc.sync.dma_start(dsta[:, 16:32, tq0:tq0 + CH1],
                                  pb16[:].rearrange("(s k) t -> s k t", s=8))
                nc.sync.dma_start(gT[:, t0:t0 + CH1], g16[:])

        # A2A #1 per token quarter
        for q in range(NQ):
            nc.gpsimd.collective_compute(
                "AllToAll", AL.bypass, replica_groups=[list(range(NCORES))],
                ins=[p_src_q[q][:].opt()], outs=[p_dst_q[q][:].opt()])

        # ---------------- Phase 3 ----------------
        with tc.tile_pool(name="p3sb", bufs=2) as sb3, \
             tc.tile_pool(name="p3out", bufs=4) as sb3o, \
             tc.tile_pool(name="p3ps", bufs=4, space="PSUM") as ps3:
            for cc in range(16):           # local triangle channel
                g = cc // CPG
                atiles, btiles = [], []
                for s in range(NCORES):    # k-tile = sender block of 96
                    at = sb3.tile([TB, N], BF16, tag="at")
                    bt = sb3.tile([TB, N], BF16, tag="bt")
                    for q in range(NQ):
                        src = p_dst_q[q][:].rearrange("(s k) (b t) -> s k b t",
                                                      s=NCORES, b=TBQ)
                        nc.sync.dma_start(at[TBQ * q:TBQ * (q + 1), :], src[s, cc])
                        nc.sync.dma_start(bt[TBQ * q:TBQ * (q + 1), :], src[s, 16 + cc])
                    atiles.append(at)
                    btiles.append(bt)
                for jt in range(6):
                    for i0, iw in ((0, 512), (512, 256)):
                        ps = ps3.tile([C, 512], F32, tag="tri")
                        for s in range(NCORES):
                            nc.tensor.matmul(
                                ps[:, :iw],
                                btiles[s][:, jt * C:(jt + 1) * C],
                                atiles[s][:, i0:i0 + iw],
                                start=(s == 0), stop=(s == NCORES - 1))
                        o16 = sb3o.tile([C, 512], BF16, tag="o16")
                        nc.vector.tensor_copy(o16[:, :iw], ps[:, :iw])
                        nc.sync.dma_start(
                            tri_src_g[g][jt * C:(jt + 1) * C, cc % CPG, i0:i0 + iw],
                            o16[:, :iw])

        # A2A #2 per channel group
        for g in range(NG):
            nc.gpsimd.collective_compute(
                "AllToAll", AL.bypass, replica_groups=[list(range(NCORES))],
                ins=[tri_src_g[g][:].opt()], outs=[tri_dst_g[g][:].opt()])

        # ---------------- Phase 4 ----------------
        with tc.tile_pool(name="p4sb", bufs=3) as sb4, \
             tc.tile_pool(name="p4ps_s", bufs=3, space="PSUM") as ps4s, \
             tc.tile_pool(name="p4ps_o", bufs=2, space="PSUM") as ps4o:
            for ci in range(NCH4):
                jl, i0 = ci // 2, (ci % 2) * CH4
                t0 = ci * CH4
                tri16 = sb4.tile([C, CH4], BF16, tag="tri16")
                for g in range(NG):
                    # partitions 16s + CPG*g + c'' <- tri_dst_g[s, jl, c'', i0:i0+CH4]
                    dstv = tri16[:].rearrange("(s r) t -> s r t", s=NCORES)[:, CPG * g:CPG * (g + 1), :]
                    nc.sync.dma_start(dstv, tri_dst_g[g][:, jl, :, i0:i0 + CH4])
                g16 = sb4.tile([C, CH4], BF16, tag="g16")
                nc.sync.dma_start(g16[:], gT[:, t0:t0 + CH4])

                sq16 = sb4.tile([C, CH4], BF16, tag="sq16")
                nc.scalar.square(sq16[:], tri16[:])
                s1 = ps4s.tile([1, CH4], F32, tag="stat")
                nc.tensor.matmul(s1[:], ones_col[:], tri16[:], start=True, stop=True)
                s2 = ps4s.tile([1, CH4], F32, tag="stat")
                nc.tensor.matmul(s2[:], ones_col[:], sq16[:], start=True, stop=True)
                ss = sb4.tile([1, CH4], F32, tag="ss")
                nc.vector.tensor_mul(ss[:], s1[:], s1[:])
                var = sb4.tile([1, CH4], F32, tag="var")
                nc.vector.tensor_tensor(var[:], s2[:], ss[:], op=AL.subtract)
                nc.vector.tensor_scalar_add(var[:], var[:], 1e-5)
                vr = sb4.tile([1, CH4], F32, tag="vr")
                nc.vector.reciprocal(vr[:], var[:])
                rstd16 = sb4.tile([1, CH4], BF16, tag="rstd16")
                nc.scalar.sqrt(rstd16[:], vr[:])
                nrsm16 = sb4.tile([1, CH4], BF16, tag="nrsm16")
                nc.vector.scalar_tensor_tensor(
                    nrsm16[:], in0=s1[:], scalar=-1.0, in1=rstd16[:],
                    op0=AL.mult, op1=AL.mult)
                bc_r = ps4s.tile([C, CH4], F32, tag="bc")
                nc.tensor.matmul(bc_r[:], ones_row[:], rstd16[:], start=True, stop=True)
                bc_n = ps4s.tile([C, CH4], F32, tag="bc")
                nc.tensor.matmul(bc_n[:], ones_row[:], nrsm16[:], start=True, stop=True)

                pso = ps4o.tile([C, CH4], F32, tag="o")
                nc.tensor.matmul(pso[:], wo_t[:], tri16[:], start=True, stop=True)

                A = sb4.tile([C, CH4], BF16, tag="A")
                nc.vector.tensor_mul(A[:], pso[:], bc_r[:])
                B = sb4.tile([C, CH4], BF16, tag="B")
                nc.vector.scalar_tensor_tensor(
                    B[:], in0=bc_n[:], scalar=wso, in1=A[:], op0=AL.mult, op1=AL.add)
                of16 = sb4.tile([C, CH4], BF16, tag="of16")
                nc.vector.scalar_tensor_tensor(
                    of16[:], in0=B[:], scalar=co, in1=g16[:], op0=AL.add, op1=AL.mult)
                nc.sync.dma_start(outT[:, t0:t0 + CH4], of16[:])

    split_excess_waits(nc)
    return nc


def host_prep(act, mask, ln1_w, ln1_b, w_proj, w_gate, ln2_w, ln2_b, w_out, w_gl):
    bf = ml_dtypes.bfloat16
    act = np.asarray(act, np.float32)
    mask = np.asarray(mask, np.float32)
    w1 = np.asarray(ln1_w, np.float32)
    b1 = np.asarray(ln1_b, np.float32)
    w2 = np.asarray(ln2_w, np.float32)
    b2 = np.asarray(ln2_b, np.float32)
    w_proj = np.asarray(w_proj, np.float32)
    w_gate = np.asarray(w_gate, np.float32)
    w_out = np.asarray(w_out, np.float32)
    w_gl = np.asarray(w_gl, np.float32)
    assert np.all(b1 == 0.0), "nonzero ln1_b not supported in proj path"

    # lhsT weights [c, d] with ln1_w folded
    def lhsT(w):
        return (w.T * w1[:, None]).astype(bf)
    wstack = np.concatenate(
        [lhsT(w_proj[:C]), lhsT(w_proj[C:]), lhsT(w_gate[:C]), lhsT(w_gate[C:]), lhsT(w_gl)],
        axis=1)
    wo_p = w_out * w2[None, :]
    woT = wo_p.T.astype(bf)
    cols = np.stack([
        w_gate[:C] @ b1, w_gate[C:] @ b1, w_gl @ b1,
        wo_p.sum(axis=1), w_out @ b2], axis=1).astype(np.float32)

    in_maps = []
    for r in range(NCORES):
        blk = act[:, TB * r:TB * (r + 1), :]        # [768 t1, 96 t2, 128 c]
        actT = np.ascontiguousarray(blk.transpose(2, 1, 0).reshape(C, TOK)).astype(bf)
        mT = np.ascontiguousarray(mask[:, TB * r:TB * (r + 1)].T.reshape(1, TOK)).astype(bf)
        in_maps.append({"actT": actT, "maskT": mT, "wstack": wstack,
                        "woT": woT, "cols": cols})
    return in_maps


def assemble(results):
    out = np.empty((N, N, C), np.float32)
    for r in range(NCORES):
        o = results[r]["outT"].astype(np.float32).reshape(C, TB, N)
        out[:, TB * r:TB * (r + 1), :] = o.transpose(2, 1, 0)
    return out


_CACHE = {}

def kernel(**inputs):
    if "nc" not in _CACHE:
        _CACHE["nc"] = build_nc()
    in_maps = host_prep(**inputs)
    r = run_bass_kernel_spmd(_CACHE["nc"], in_maps, core_ids=list(range(NCORES)))
    return assemble(r.results)
